# revision 24
# baseline (speedup 1.0000x reference)
"""Multi-head causal self-attention on 8 TRN2 NeuronCores.

Problem: B=2, T=4096, D=512, H=8 heads (hd=64), fp32 in/out.

Sharding: core c in 0..7 handles batch b = c//4 and head pair g = c%4
(heads 2g, 2g+1 -> D-slice [128g, 128g+128)). Each core computes
    partial_out = concat_h( softmax(causal(Q_h K_h^T / 8)) V_h ) @ W_O[slice]
for its two heads; the host sums the 4 partials per batch and adds b_O.

On-core dataflow (all matmul operands bf16, f32 PSUM accumulation):
  - X^T (host-pretransposed) streams in as 4 [128,4096] chunks.
  - Q^T,K^T [128(d-pair),4096] = W_chunk^T @ X^T, bias added during the
    PSUM->SBUF bf16 copy (per-partition scalar add on DVE).
  - V [4096,64+1] per head in natural layout (ones column appended ->
    the attention row-sum L falls out of the PV matmul for free).
  - Scores are computed transposed, S^T[k-block, q] (contraction over the
    64-dim head axis; the two heads run in disjoint PE row groups), causally
    streamed: for key block kb only q >= 128*kb is computed. exp() runs on
    ScalarE straight out of PSUM with the 1/8 scale folded in; the diagonal
    128x128 subtile is masked by accumulating -1e9 upper-triangle via an
    identity matmul before the exp.
  - Z^T_aug[65, q] accumulates P^T-block x V_aug over key blocks in PSUM;
    row 64 is L. Normalisation: reciprocal of L (DVE), broadcast across 64
    partitions (SBUF->SBUF DMA), one tensor-tensor multiply.
  - O-projection: lhsT = stacked [Z_A; Z_B] [128, t-tile], rhs = W_O pair
    [128,512]; per-core partial written straight to HBM.
"""

import numpy as np

import concourse.bass as bass
import concourse.mybir as mybir
from concourse.tile import TileContext
from concourse.bass_utils import run_bass_kernel_spmd

try:
    import ml_dtypes

    _BF16 = ml_dtypes.bfloat16
except ImportError:  # pragma: no cover
    _BF16 = None

F32 = mybir.dt.float32
BF16 = mybir.dt.bfloat16

B, T, D, H = 2, 4096, 512, 8
HD = D // H  # 64
SW = 512  # q-slice width
NS = T // SW  # 8 q-slices
NKC = D // 128  # 4 contraction chunks for the projections
NTT = T // 128  # 32 t-tiles / key blocks
GK = 2  # key blocks grouped per exp() call (2 PSUM banks)
NEG = -1.0e9


def _split_waits(nc, max_waits=1):
    """The staged walrus rejects >1 semaphore wait per instruction; hoist
    extras onto same-engine NoOps inserted right before the instruction."""
    counter = 0
    for f in nc.m.functions:
        for blk in f.blocks:
            insts = blk.instructions
            out, changed = [], False
            for ins in insts:
                si = getattr(ins, "sync_info", None)
                waits = list(si.on_wait) if si is not None and si.on_wait else []
                if len(waits) > max_waits:
                    changed = True
                    for w in waits[:-max_waits]:
                        counter += 1
                        nop = mybir.InstNoOp(
                            name=f"I-wsplit-{counter}",
                            engine=ins.engine,
                            ins=[],
                            outs=[],
                        )
                        nop.sync_info = mybir.SyncInfo(on_wait=[w], on_update=[])
                        out.append(nop)
                    ins.sync_info = mybir.SyncInfo(
                        on_wait=waits[-max_waits:], on_update=list(si.on_update)
                    )
                out.append(ins)
            if changed:
                blk.instructions = out
    return counter


def build_nc():
    nc = bass.Bass("TRN2")

    xt = nc.dram_tensor("xt", [D, T], BF16, kind="ExternalInput")
    wq = nc.dram_tensor("wq", [D, 128], BF16, kind="ExternalInput")
    wk = nc.dram_tensor("wk", [D, 128], BF16, kind="ExternalInput")
    wv = nc.dram_tensor("wv", [D, 128], BF16, kind="ExternalInput")
    wo = nc.dram_tensor("wo", [128, D], BF16, kind="ExternalInput")
    bq = nc.dram_tensor("bq", [128, 1], F32, kind="ExternalInput")
    bk = nc.dram_tensor("bk", [128, 1], F32, kind="ExternalInput")
    bv = nc.dram_tensor("bv", [1, 128], BF16, kind="ExternalInput")
    out = nc.dram_tensor("out", [T, D], F32, kind="ExternalOutput")

    ident_np = np.eye(128, dtype=np.float32)
    # maskneg[k, q'] = 0 where q' >= k else NEG  (S^T diagonal subtile mask)
    mask_np = np.where(
        np.arange(128)[None, :] >= np.arange(128)[:, None], 0.0, NEG
    ).astype(np.float32)
    ident_dram = nc.inline_tensor(ident_np.astype(_BF16), name="identc")
    mask_dram = nc.inline_tensor(mask_np.astype(_BF16), name="maskc")

    with TileContext(nc) as tc:
        with (
            tc.tile_pool(name="singles", bufs=1) as singles,
            tc.tile_pool(name="ps", bufs=3, space="PSUM") as ps,
            tc.tile_pool(name="zps", bufs=1, space="PSUM") as zps,
            tc.tile_pool(name="pt", bufs=6) as ptp,
            tc.tile_pool(name="sl", bufs=2) as slp,
            tc.tile_pool(name="outp", bufs=3) as outp,
            tc.tile_pool(name="drp", bufs=2, space="DRAM") as drp,
        ):
            # ---- static SBUF ----
            xt_sb = [
                [
                    singles.tile(
                        [128, SW], BF16, tag=f"xt{c}_{s}", name=f"xt_sb{c}_{s}"
                    )
                    for s in range(NS)
                ]
                for c in range(NKC)
            ]
            for s in range(NS):
                for c in range(NKC):
                    nc.sync.dma_start(
                        out=xt_sb[c][s][:, :],
                        in_=xt[c * 128 : (c + 1) * 128, s * SW : (s + 1) * SW],
                    )

            wq_sb = singles.tile([128, NKC, 128], BF16, tag="wq")
            wk_sb = singles.tile([128, NKC, 128], BF16, tag="wk")
            wv_sb = singles.tile([128, NKC, 128], BF16, tag="wv")
            for c in range(NKC):
                nc.sync.dma_start(out=wq_sb[:, c, :], in_=wq[c * 128 : (c + 1) * 128, :])
                nc.sync.dma_start(out=wk_sb[:, c, :], in_=wk[c * 128 : (c + 1) * 128, :])
                nc.sync.dma_start(out=wv_sb[:, c, :], in_=wv[c * 128 : (c + 1) * 128, :])
            wo_sb = singles.tile([128, D], BF16, tag="wo")
            nc.sync.dma_start(out=wo_sb[:, :], in_=wo[:, :])

            bq_sb = singles.tile([128, 1], F32, tag="bq")
            bk_sb = singles.tile([128, 1], F32, tag="bk")
            bv_sb = singles.tile([1, 512], BF16, tag="bv")
            nc.sync.dma_start(out=bq_sb[:, :], in_=bq[:, :])
            nc.sync.dma_start(out=bk_sb[:, :], in_=bk[:, :])
            for j in range(4):
                nc.sync.dma_start(out=bv_sb[:, j * 128 : (j + 1) * 128], in_=bv[:, :])

            ident_sb = singles.tile([128, 128], BF16, tag="ident")
            mask_sb = singles.tile([128, 128], BF16, tag="mask")
            nc.sync.dma_start(out=ident_sb[:, :], in_=ident_dram[:, :])
            nc.sync.dma_start(out=mask_sb[:, :], in_=mask_dram[:, :])
            ones_sb = singles.tile([1, 128], BF16, tag="ones")
            nc.vector.memset(ones_sb[:, :], 1.0)

            qt_sb = [
                singles.tile([128, SW], BF16, tag=f"qt{s}", name=f"qt_sb{s}")
                for s in range(NS)
            ]
            kt_sb = [
                singles.tile([128, SW], BF16, tag=f"kt{s}", name=f"kt_sb{s}")
                for s in range(NS)
            ]
            # V_aug per head per key block: [128(t), 65]; col 64 = ones
            va_sb = [
                singles.tile([128, HD + 1], BF16, tag=f"va{t}", name=f"va_sb{t}")
                for t in range(NTT)
            ]
            vb_sb = [
                singles.tile([128, HD + 1], BF16, tag=f"vb{t}", name=f"vb_sb{t}")
                for t in range(NTT)
            ]
            ones_row = singles.tile([1, SW], F32, tag="onesrow")
            nc.vector.memset(ones_row[:, :], 1.0)

            # ---- QKV projections (emitted per q-slice, interleaved with
            # attention so ScalarE starts exp-ing early) ----
            def emit_qkv(s):
                cols = slice(s * SW, (s + 1) * SW)
                ps_q = ps.tile([128, SW], F32, tag="sg", name="ps_q")
                for c in range(NKC):
                    nc.tensor.matmul(
                        ps_q[:, :],
                        lhsT=wq_sb[:, c, :],
                        rhs=xt_sb[c][s][:, :],
                        start=(c == 0),
                        stop=(c == NKC - 1),
                        skip_group_check=True,
                    )
                nc.vector.tensor_scalar_add(qt_sb[s][:, :], ps_q[:, :], bq_sb[:, :])
                ps_k = ps.tile([128, SW], F32, tag="sg", name="ps_k")
                for c in range(NKC):
                    nc.tensor.matmul(
                        ps_k[:, :],
                        lhsT=wk_sb[:, c, :],
                        rhs=xt_sb[c][s][:, :],
                        start=(c == 0),
                        stop=(c == NKC - 1),
                        skip_group_check=True,
                    )
                nc.vector.tensor_scalar_add(kt_sb[s][:, :], ps_k[:, :], bk_sb[:, :])
                for t in range(4 * s, 4 * s + 4):
                    tloc = slice((t % 4) * 128, (t % 4 + 1) * 128)
                    ps_v = ps.tile([128, 128], F32, tag="sg", name="ps_v")
                    for c in range(NKC):
                        nc.tensor.matmul(
                            ps_v[:, :],
                            lhsT=xt_sb[c][s][:, tloc],
                            rhs=wv_sb[:, c, :],
                            start=(c == 0),
                            stop=False,
                            skip_group_check=True,
                        )
                    # + b_V broadcast over rows:  ones[1,128]^T @ bv[1,128]
                    nc.tensor.matmul(
                        ps_v[:, :],
                        lhsT=ones_sb[:, :],
                        rhs=bv_sb[:, 0:128],
                        start=False,
                        stop=True,
                        skip_group_check=True,
                    )
                    nc.vector.tensor_copy(va_sb[t][:, 0:HD], ps_v[:, 0:HD])
                    nc.vector.tensor_copy(vb_sb[t][:, 0:HD], ps_v[:, HD:128])
                    nc.vector.memset(va_sb[t][:, HD : HD + 1], 1.0)
                    nc.vector.memset(vb_sb[t][:, HD : HD + 1], 1.0)

            # ---- attention ----
            vmat = (va_sb, vb_sb)
            hrows = (slice(0, HD), slice(HD, 128))

            def emit_oproj(znpair_t, qs_t):
                for j in range(4):
                    ps_o = ps.tile([128, D], F32, tag="sg", name="ps_o")
                    nc.tensor.matmul(
                        ps_o[:, :],
                        lhsT=znpair_t[:, j * 128 : (j + 1) * 128],
                        rhs=wo_sb[:, :],
                        start=True,
                        stop=True,
                        skip_group_check=True,
                    )
                    o_sb = outp.tile([128, D], F32, tag="ot", name="o_sb")
                    nc.vector.tensor_copy(o_sb[:, :], ps_o[:, :])
                    r0 = qs_t + j * 128
                    nc.sync.dma_start(out=out[r0 : r0 + 128, :], in_=o_sb[:, :])

            pending = None
            for s in range(NS):
                emit_qkv(s)
                qs = s * SW
                nkb = 4 * (s + 1)
                zaug = [
                    zps.tile([HD + 1, SW], F32, tag="za", name="zauga"),
                    zps.tile([HD + 1, SW], F32, tag="zb", name="zaugb"),
                ]
                # pack key blocks tightly into groups; a matmul output may
                # not cross a PSUM bank boundary, so bump to the next bank
                # when a block would straddle one
                groups, cur, cur_cols = [], [], 0
                for kb in range(nkb):
                    qlo = max(qs, kb * 128)
                    n = qs + SW - qlo
                    off = cur_cols
                    if off % SW + n > SW:
                        off = ((off + SW - 1) // SW) * SW
                    if off + n > GK * SW:
                        groups.append(cur)
                        cur, off = [], 0
                    cur.append((kb, off, n, qlo))
                    cur_cols = off + n
                if cur:
                    groups.append(cur)
                def emit_av(av):
                    pt_t, grp_t = av
                    for h in range(2):
                        for kb, off, n, qlo in grp_t:
                            nc.tensor.matmul(
                                zaug[h][0 : HD + 1, qlo - qs : SW],
                                lhsT=vmat[h][kb][:, :],
                                rhs=pt_t[h][:, off : off + n],
                                start=(kb == 0),
                                stop=(kb == nkb - 1),
                                skip_group_check=True,
                            )

                av_queue = []
                for grp in groups:
                    used = grp[-1][1] + grp[-1][2]
                    sg = [None, None]
                    pt = [None, None]
                    for h in range(2):
                        sg[h] = ps.tile([128, GK * SW], F32, tag="sg", name="sg")
                        pt[h] = ptp.tile([128, GK * SW], BF16, tag="pt", name="pt")
                    # scores (both heads interleaved -> disjoint PE row groups)
                    for kb, off, n, qlo in grp:
                        diag = kb * 128 >= qs
                        for h in range(2):
                            nc.tensor.matmul(
                                sg[h][:, off : off + n],
                                lhsT=kt_sb[kb // 4][hrows[h], (kb % 4) * 128 : (kb % 4 + 1) * 128],
                                rhs=qt_sb[s][hrows[h], qlo - qs : qlo - qs + n],
                                start=True,
                                stop=not diag,
                                skip_group_check=True,
                                tile_position=(h * HD, 0),
                            )
                        if diag:
                            for h in range(2):
                                nc.tensor.matmul(
                                    sg[h][:, off : off + 128],
                                    lhsT=ident_sb[:, :],
                                    rhs=mask_sb[:, :],
                                    start=False,
                                    stop=True,
                                    skip_group_check=True,
                                )
                    for h in range(2):
                        nc.scalar.activation(
                            out=pt[h][:, 0:used],
                            in_=sg[h][:, 0:used],
                            func=mybir.ActivationFunctionType.Exp,
                            scale=0.125,
                        )
                    av_queue.append((pt, grp))
                    if len(av_queue) > 1:
                        emit_av(av_queue.pop(0))
                while av_queue:
                    emit_av(av_queue.pop(0))

                # previous slice's O-projection: its normalisation chain has
                # had a whole slice of compute to finish -> PE never stalls
                if pending is not None:
                    emit_oproj(*pending)
                    pending = None

                # evacuate Z^T_aug to SBUF right away (frees the PSUM bank);
                # L row lands at partition 0 so GpSimd ops are partition-aligned
                zsb = [None, None]
                lrow = [None, None]
                for h in range(2):
                    zsb[h] = slp.tile([HD, SW], F32, tag=f"zsb{h}", name="zsb")
                    nc.vector.tensor_copy(zsb[h][:, :], zaug[h][0:HD, :])
                    lrow[h] = slp.tile([1, SW], F32, tag=f"lr{h}", name="lrow")
                    nc.vector.tensor_copy(lrow[h][:, :], zaug[h][HD : HD + 1, :])

                # normalise z[:, q] / L[q]; the reciprocal runs on a
                # [128, 4] partition-spread layout (DVE iterative divide
                # costs free-dim x 8 cycles, so spread the 512 elements)
                znpair = slp.tile([128, SW], BF16, tag="zn")
                znb = slp.tile([HD, SW], BF16, tag="znb")
                for h in range(2):
                    rd = drp.tile([1, SW], F32, tag=f"rd{h}", name="rd")
                    nc.sync.dma_start(out=rd[:, :], in_=lrow[h][:, :])
                    lsp = slp.tile([128, SW // 128], F32, tag=f"lsp{h}", name="lsp")
                    nc.sync.dma_start(
                        out=lsp[:, :],
                        in_=rd[0, :].rearrange("(p f) -> p f", p=128),
                    )
                    rsp = slp.tile([128, SW // 128], F32, tag=f"rsp{h}", name="rsp")
                    nc.vector.reciprocal(rsp[:, :], lsp[:, :])
                    rd2 = drp.tile([1, SW], F32, tag=f"rd2{h}", name="rd2")
                    nc.sync.dma_start(
                        out=rd2[0, :].rearrange("(p f) -> p f", p=128),
                        in_=rsp[:, :],
                    )
                    bc = slp.tile([HD, SW], F32, tag=f"bc{h}")
                    rap = rd2[:, :]
                    bcast_src = bass.AP(
                        tensor=rap.tensor,
                        offset=rap.offset,
                        ap=[[0, HD]] + list(rap.ap[1:]),
                    )
                    nc.sync.dma_start(out=bc[:, :], in_=bcast_src)
                    dst = znpair[0:HD, :] if h == 0 else znb[:, :]
                    nc.vector.tensor_mul(dst, zsb[h][:, :], bc[:, :])
                # move head B rows into partitions 64..127
                nc.gpsimd.dma_start(out=znpair[HD:128, :], in_=znb[:, :])
                pending = (znpair, qs)

            if pending is not None:
                emit_oproj(*pending)

    _split_waits(nc)
    return nc


_NC_CACHE = {}


def _get_nc():
    if "nc" not in _NC_CACHE:
        _NC_CACHE["nc"] = build_nc()
    return _NC_CACHE["nc"]


def make_in_maps(combined_embed, W_K, b_K, W_Q, b_Q, W_V, b_V, W_O, b_O):
    f32 = np.float32
    in_maps = []
    for c in range(8):
        b = c // 4
        g = c % 4
        sl = slice(g * 128, (g + 1) * 128)
        xt = np.ascontiguousarray(np.asarray(combined_embed[b], f32).T)
        in_maps.append(
            {
                "xt": xt.astype(_BF16),
                "wq": np.ascontiguousarray(np.asarray(W_Q, f32)[:, sl]).astype(_BF16),
                "wk": np.ascontiguousarray(np.asarray(W_K, f32)[:, sl]).astype(_BF16),
                "wv": np.ascontiguousarray(np.asarray(W_V, f32)[:, sl]).astype(_BF16),
                "wo": np.ascontiguousarray(np.asarray(W_O, f32)[sl, :]).astype(_BF16),
                "bq": np.asarray(b_Q, f32)[sl].reshape(128, 1).copy(),
                "bk": np.asarray(b_K, f32)[sl].reshape(128, 1).copy(),
                "bv": np.asarray(b_V, f32)[sl].reshape(1, 128).astype(_BF16),
            }
        )
    return in_maps


def run_cores(in_maps, **kwargs):
    nc = _get_nc()
    return run_bass_kernel_spmd(nc, in_maps, core_ids=list(range(8)), **kwargs)


def kernel(
    combined_embed, W_K, b_K, W_Q, b_Q, W_V, b_V, W_O, b_O
):  # full inputs -> full output
    in_maps = make_in_maps(
        combined_embed, W_K, b_K, W_Q, b_Q, W_V, b_V, W_O, b_O
    )
    res = run_cores(in_maps)
    out = np.zeros((B, T, D), np.float32)
    for c in range(8):
        out[c // 4] += res.results[c]["out"]
    out += np.asarray(b_O, np.float32)[None, None, :]
    return out


# revision 25
# speedup vs baseline: 1.0031x; 1.0031x over previous
"""Multi-head causal self-attention on 8 TRN2 NeuronCores.

Problem: B=2, T=4096, D=512, H=8 heads (hd=64), fp32 in/out.

Sharding: core c in 0..7 handles batch b = c//4 and head pair g = c%4
(heads 2g, 2g+1 -> D-slice [128g, 128g+128)). Each core computes
    partial_out = concat_h( softmax(causal(Q_h K_h^T / 8)) V_h ) @ W_O[slice]
for its two heads; the host sums the 4 partials per batch and adds b_O.

On-core dataflow (all matmul operands bf16, f32 PSUM accumulation):
  - X^T (host-pretransposed) streams in as 4 [128,4096] chunks.
  - Q^T,K^T [128(d-pair),4096] = W_chunk^T @ X^T, bias added during the
    PSUM->SBUF bf16 copy (per-partition scalar add on DVE).
  - V [4096,64+1] per head in natural layout (ones column appended ->
    the attention row-sum L falls out of the PV matmul for free).
  - Scores are computed transposed, S^T[k-block, q] (contraction over the
    64-dim head axis; the two heads run in disjoint PE row groups), causally
    streamed: for key block kb only q >= 128*kb is computed. exp() runs on
    ScalarE straight out of PSUM with the 1/8 scale folded in; the diagonal
    128x128 subtile is masked by accumulating -1e9 upper-triangle via an
    identity matmul before the exp.
  - Z^T_aug[65, q] accumulates P^T-block x V_aug over key blocks in PSUM;
    row 64 is L. Normalisation: reciprocal of L (DVE), broadcast across 64
    partitions (SBUF->SBUF DMA), one tensor-tensor multiply.
  - O-projection: lhsT = stacked [Z_A; Z_B] [128, t-tile], rhs = W_O pair
    [128,512]; per-core partial written straight to HBM.
"""

import numpy as np

import concourse.bass as bass
import concourse.mybir as mybir
from concourse.tile import TileContext
from concourse.bass_utils import run_bass_kernel_spmd

try:
    import ml_dtypes

    _BF16 = ml_dtypes.bfloat16
except ImportError:  # pragma: no cover
    _BF16 = None

F32 = mybir.dt.float32
BF16 = mybir.dt.bfloat16

B, T, D, H = 2, 4096, 512, 8
HD = D // H  # 64
SW = 512  # q-slice width
NS = T // SW  # 8 q-slices
NKC = D // 128  # 4 contraction chunks for the projections
NTT = T // 128  # 32 t-tiles / key blocks
GK = 2  # key blocks grouped per exp() call (2 PSUM banks)
NEG = -1.0e9


def _split_waits(nc, max_waits=1):
    """The staged walrus rejects >1 semaphore wait per instruction; hoist
    extras onto same-engine NoOps inserted right before the instruction."""
    counter = 0
    for f in nc.m.functions:
        for blk in f.blocks:
            insts = blk.instructions
            out, changed = [], False
            for ins in insts:
                si = getattr(ins, "sync_info", None)
                waits = list(si.on_wait) if si is not None and si.on_wait else []
                if len(waits) > max_waits:
                    changed = True
                    for w in waits[:-max_waits]:
                        counter += 1
                        nop = mybir.InstNoOp(
                            name=f"I-wsplit-{counter}",
                            engine=ins.engine,
                            ins=[],
                            outs=[],
                        )
                        nop.sync_info = mybir.SyncInfo(on_wait=[w], on_update=[])
                        out.append(nop)
                    ins.sync_info = mybir.SyncInfo(
                        on_wait=waits[-max_waits:], on_update=list(si.on_update)
                    )
                out.append(ins)
            if changed:
                blk.instructions = out
    return counter


def build_nc():
    nc = bass.Bass("TRN2")

    xt = nc.dram_tensor("xt", [D, T], BF16, kind="ExternalInput")
    wq = nc.dram_tensor("wq", [D, 128], BF16, kind="ExternalInput")
    wk = nc.dram_tensor("wk", [D, 128], BF16, kind="ExternalInput")
    wv = nc.dram_tensor("wv", [D, 128], BF16, kind="ExternalInput")
    wo = nc.dram_tensor("wo", [128, D], BF16, kind="ExternalInput")
    bq = nc.dram_tensor("bq", [128, 1], F32, kind="ExternalInput")
    bk = nc.dram_tensor("bk", [128, 1], F32, kind="ExternalInput")
    bv = nc.dram_tensor("bv", [1, 128], BF16, kind="ExternalInput")
    out = nc.dram_tensor("out", [T, D], F32, kind="ExternalOutput")

    ident_np = np.eye(128, dtype=np.float32)
    # maskneg[k, q'] = 0 where q' >= k else NEG  (S^T diagonal subtile mask)
    mask_np = np.where(
        np.arange(128)[None, :] >= np.arange(128)[:, None], 0.0, NEG
    ).astype(np.float32)
    ident_dram = nc.inline_tensor(ident_np.astype(_BF16), name="identc")
    mask_dram = nc.inline_tensor(mask_np.astype(_BF16), name="maskc")

    with TileContext(nc) as tc:
        with (
            tc.tile_pool(name="singles", bufs=1) as singles,
            tc.tile_pool(name="ps", bufs=3, space="PSUM") as ps,
            tc.tile_pool(name="zps", bufs=1, space="PSUM") as zps,
            tc.tile_pool(name="pt", bufs=6) as ptp,
            tc.tile_pool(name="sl", bufs=2) as slp,
            tc.tile_pool(name="outp", bufs=3) as outp,
            tc.tile_pool(name="drp", bufs=2, space="DRAM") as drp,
        ):
            # ---- static SBUF ----
            xt_sb = [
                [
                    singles.tile(
                        [128, SW], BF16, tag=f"xt{c}_{s}", name=f"xt_sb{c}_{s}"
                    )
                    for s in range(NS)
                ]
                for c in range(NKC)
            ]
            for c in range(NKC):
                nc.sync.dma_start(
                    out=xt_sb[c][0][:, :],
                    in_=xt[c * 128 : (c + 1) * 128, 0:SW],
                )
            for c in range(NKC):
                for s in range(1, NS):
                    nc.sync.dma_start(
                        out=xt_sb[c][s][:, :],
                        in_=xt[c * 128 : (c + 1) * 128, s * SW : (s + 1) * SW],
                    )

            wq_sb = singles.tile([128, NKC, 128], BF16, tag="wq")
            wk_sb = singles.tile([128, NKC, 128], BF16, tag="wk")
            wv_sb = singles.tile([128, NKC, 128], BF16, tag="wv")
            for c in range(NKC):
                nc.sync.dma_start(out=wq_sb[:, c, :], in_=wq[c * 128 : (c + 1) * 128, :])
                nc.sync.dma_start(out=wk_sb[:, c, :], in_=wk[c * 128 : (c + 1) * 128, :])
                nc.sync.dma_start(out=wv_sb[:, c, :], in_=wv[c * 128 : (c + 1) * 128, :])
            wo_sb = singles.tile([128, D], BF16, tag="wo")
            nc.sync.dma_start(out=wo_sb[:, :], in_=wo[:, :])

            bq_sb = singles.tile([128, 1], F32, tag="bq")
            bk_sb = singles.tile([128, 1], F32, tag="bk")
            bv_sb = singles.tile([1, 512], BF16, tag="bv")
            nc.sync.dma_start(out=bq_sb[:, :], in_=bq[:, :])
            nc.sync.dma_start(out=bk_sb[:, :], in_=bk[:, :])
            for j in range(4):
                nc.sync.dma_start(out=bv_sb[:, j * 128 : (j + 1) * 128], in_=bv[:, :])

            ident_sb = singles.tile([128, 128], BF16, tag="ident")
            mask_sb = singles.tile([128, 128], BF16, tag="mask")
            nc.sync.dma_start(out=ident_sb[:, :], in_=ident_dram[:, :])
            nc.sync.dma_start(out=mask_sb[:, :], in_=mask_dram[:, :])
            ones_sb = singles.tile([1, 128], BF16, tag="ones")
            nc.vector.memset(ones_sb[:, :], 1.0)

            qt_sb = [
                singles.tile([128, SW], BF16, tag=f"qt{s}", name=f"qt_sb{s}")
                for s in range(NS)
            ]
            kt_sb = [
                singles.tile([128, SW], BF16, tag=f"kt{s}", name=f"kt_sb{s}")
                for s in range(NS)
            ]
            # V_aug per head per key block: [128(t), 65]; col 64 = ones
            va_sb = [
                singles.tile([128, HD + 1], BF16, tag=f"va{t}", name=f"va_sb{t}")
                for t in range(NTT)
            ]
            vb_sb = [
                singles.tile([128, HD + 1], BF16, tag=f"vb{t}", name=f"vb_sb{t}")
                for t in range(NTT)
            ]
            ones_row = singles.tile([1, SW], F32, tag="onesrow")
            nc.vector.memset(ones_row[:, :], 1.0)

            # ---- QKV projections (emitted per q-slice, interleaved with
            # attention so ScalarE starts exp-ing early) ----
            def emit_qkv(s):
                cols = slice(s * SW, (s + 1) * SW)
                ps_q = ps.tile([128, SW], F32, tag="sg", name="ps_q")
                for c in range(NKC):
                    nc.tensor.matmul(
                        ps_q[:, :],
                        lhsT=wq_sb[:, c, :],
                        rhs=xt_sb[c][s][:, :],
                        start=(c == 0),
                        stop=(c == NKC - 1),
                        skip_group_check=True,
                    )
                nc.vector.tensor_scalar_add(qt_sb[s][:, :], ps_q[:, :], bq_sb[:, :])
                ps_k = ps.tile([128, SW], F32, tag="sg", name="ps_k")
                for c in range(NKC):
                    nc.tensor.matmul(
                        ps_k[:, :],
                        lhsT=wk_sb[:, c, :],
                        rhs=xt_sb[c][s][:, :],
                        start=(c == 0),
                        stop=(c == NKC - 1),
                        skip_group_check=True,
                    )
                nc.vector.tensor_scalar_add(kt_sb[s][:, :], ps_k[:, :], bk_sb[:, :])
                for t in range(4 * s, 4 * s + 4):
                    tloc = slice((t % 4) * 128, (t % 4 + 1) * 128)
                    ps_v = ps.tile([128, 128], F32, tag="sg", name="ps_v")
                    for c in range(NKC):
                        nc.tensor.matmul(
                            ps_v[:, :],
                            lhsT=xt_sb[c][s][:, tloc],
                            rhs=wv_sb[:, c, :],
                            start=(c == 0),
                            stop=False,
                            skip_group_check=True,
                        )
                    # + b_V broadcast over rows:  ones[1,128]^T @ bv[1,128]
                    nc.tensor.matmul(
                        ps_v[:, :],
                        lhsT=ones_sb[:, :],
                        rhs=bv_sb[:, 0:128],
                        start=False,
                        stop=True,
                        skip_group_check=True,
                    )
                    nc.vector.tensor_copy(va_sb[t][:, 0:HD], ps_v[:, 0:HD])
                    nc.vector.tensor_copy(vb_sb[t][:, 0:HD], ps_v[:, HD:128])
                    nc.vector.memset(va_sb[t][:, HD : HD + 1], 1.0)
                    nc.vector.memset(vb_sb[t][:, HD : HD + 1], 1.0)

            # ---- attention ----
            vmat = (va_sb, vb_sb)
            hrows = (slice(0, HD), slice(HD, 128))

            def emit_oproj(znpair_t, qs_t):
                for j in range(4):
                    ps_o = ps.tile([128, D], F32, tag="sg", name="ps_o")
                    nc.tensor.matmul(
                        ps_o[:, :],
                        lhsT=znpair_t[:, j * 128 : (j + 1) * 128],
                        rhs=wo_sb[:, :],
                        start=True,
                        stop=True,
                        skip_group_check=True,
                    )
                    o_sb = outp.tile([128, D], F32, tag="ot", name="o_sb")
                    nc.vector.tensor_copy(o_sb[:, :], ps_o[:, :])
                    r0 = qs_t + j * 128
                    nc.sync.dma_start(out=out[r0 : r0 + 128, :], in_=o_sb[:, :])

            pending = None
            for s in range(NS):
                emit_qkv(s)
                qs = s * SW
                nkb = 4 * (s + 1)
                zaug = [
                    zps.tile([HD + 1, SW], F32, tag="za", name="zauga"),
                    zps.tile([HD + 1, SW], F32, tag="zb", name="zaugb"),
                ]
                # pack key blocks tightly into groups; a matmul output may
                # not cross a PSUM bank boundary, so bump to the next bank
                # when a block would straddle one
                groups, cur, cur_cols = [], [], 0
                for kb in range(nkb):
                    qlo = max(qs, kb * 128)
                    n = qs + SW - qlo
                    off = cur_cols
                    if off % SW + n > SW:
                        off = ((off + SW - 1) // SW) * SW
                    if off + n > GK * SW:
                        groups.append(cur)
                        cur, off = [], 0
                    cur.append((kb, off, n, qlo))
                    cur_cols = off + n
                if cur:
                    groups.append(cur)
                def emit_av(av):
                    pt_t, grp_t = av
                    for h in range(2):
                        for kb, off, n, qlo in grp_t:
                            nc.tensor.matmul(
                                zaug[h][0 : HD + 1, qlo - qs : SW],
                                lhsT=vmat[h][kb][:, :],
                                rhs=pt_t[h][:, off : off + n],
                                start=(kb == 0),
                                stop=(kb == nkb - 1),
                                skip_group_check=True,
                            )

                av_queue = []
                for grp in groups:
                    used = grp[-1][1] + grp[-1][2]
                    sg = [None, None]
                    pt = [None, None]
                    for h in range(2):
                        sg[h] = ps.tile([128, GK * SW], F32, tag="sg", name="sg")
                        pt[h] = ptp.tile([128, GK * SW], BF16, tag="pt", name="pt")
                    # scores (both heads interleaved -> disjoint PE row groups)
                    for kb, off, n, qlo in grp:
                        diag = kb * 128 >= qs
                        for h in range(2):
                            nc.tensor.matmul(
                                sg[h][:, off : off + n],
                                lhsT=kt_sb[kb // 4][hrows[h], (kb % 4) * 128 : (kb % 4 + 1) * 128],
                                rhs=qt_sb[s][hrows[h], qlo - qs : qlo - qs + n],
                                start=True,
                                stop=not diag,
                                skip_group_check=True,
                                tile_position=(h * HD, 0),
                            )
                        if diag:
                            for h in range(2):
                                nc.tensor.matmul(
                                    sg[h][:, off : off + 128],
                                    lhsT=ident_sb[:, :],
                                    rhs=mask_sb[:, :],
                                    start=False,
                                    stop=True,
                                    skip_group_check=True,
                                )
                    for h in range(2):
                        nc.scalar.activation(
                            out=pt[h][:, 0:used],
                            in_=sg[h][:, 0:used],
                            func=mybir.ActivationFunctionType.Exp,
                            scale=0.125,
                        )
                    av_queue.append((pt, grp))
                    if len(av_queue) > 1:
                        emit_av(av_queue.pop(0))
                while av_queue:
                    emit_av(av_queue.pop(0))

                # previous slice's O-projection: its normalisation chain has
                # had a whole slice of compute to finish -> PE never stalls
                if pending is not None:
                    emit_oproj(*pending)
                    pending = None

                # evacuate Z^T_aug to SBUF right away (frees the PSUM bank);
                # L row lands at partition 0 so GpSimd ops are partition-aligned
                zsb = [None, None]
                lrow = [None, None]
                for h in range(2):
                    zsb[h] = slp.tile([HD, SW], F32, tag=f"zsb{h}", name="zsb")
                    nc.vector.tensor_copy(zsb[h][:, :], zaug[h][0:HD, :])
                    lrow[h] = slp.tile([1, SW], F32, tag=f"lr{h}", name="lrow")
                    nc.vector.tensor_copy(lrow[h][:, :], zaug[h][HD : HD + 1, :])

                # normalise z[:, q] / L[q]; the reciprocal runs on a
                # [128, 4] partition-spread layout (DVE iterative divide
                # costs free-dim x 8 cycles, so spread the 512 elements)
                znpair = slp.tile([128, SW], BF16, tag="zn")
                znb = slp.tile([HD, SW], BF16, tag="znb")
                for h in range(2):
                    rd = drp.tile([1, SW], F32, tag=f"rd{h}", name="rd")
                    nc.sync.dma_start(out=rd[:, :], in_=lrow[h][:, :])
                    lsp = slp.tile([128, SW // 128], F32, tag=f"lsp{h}", name="lsp")
                    nc.sync.dma_start(
                        out=lsp[:, :],
                        in_=rd[0, :].rearrange("(p f) -> p f", p=128),
                    )
                    rsp = slp.tile([128, SW // 128], F32, tag=f"rsp{h}", name="rsp")
                    nc.vector.reciprocal(rsp[:, :], lsp[:, :])
                    rd2 = drp.tile([1, SW], F32, tag=f"rd2{h}", name="rd2")
                    nc.sync.dma_start(
                        out=rd2[0, :].rearrange("(p f) -> p f", p=128),
                        in_=rsp[:, :],
                    )
                    bc = slp.tile([HD, SW], F32, tag=f"bc{h}")
                    rap = rd2[:, :]
                    bcast_src = bass.AP(
                        tensor=rap.tensor,
                        offset=rap.offset,
                        ap=[[0, HD]] + list(rap.ap[1:]),
                    )
                    nc.sync.dma_start(out=bc[:, :], in_=bcast_src)
                    dst = znpair[0:HD, :] if h == 0 else znb[:, :]
                    nc.vector.tensor_mul(dst, zsb[h][:, :], bc[:, :])
                # move head B rows into partitions 64..127
                nc.gpsimd.dma_start(out=znpair[HD:128, :], in_=znb[:, :])
                pending = (znpair, qs)

            if pending is not None:
                emit_oproj(*pending)

    _split_waits(nc)
    return nc


_NC_CACHE = {}


def _get_nc():
    if "nc" not in _NC_CACHE:
        _NC_CACHE["nc"] = build_nc()
    return _NC_CACHE["nc"]


def make_in_maps(combined_embed, W_K, b_K, W_Q, b_Q, W_V, b_V, W_O, b_O):
    f32 = np.float32
    in_maps = []
    for c in range(8):
        b = c // 4
        g = c % 4
        sl = slice(g * 128, (g + 1) * 128)
        xt = np.ascontiguousarray(np.asarray(combined_embed[b], f32).T)
        in_maps.append(
            {
                "xt": xt.astype(_BF16),
                "wq": np.ascontiguousarray(np.asarray(W_Q, f32)[:, sl]).astype(_BF16),
                "wk": np.ascontiguousarray(np.asarray(W_K, f32)[:, sl]).astype(_BF16),
                "wv": np.ascontiguousarray(np.asarray(W_V, f32)[:, sl]).astype(_BF16),
                "wo": np.ascontiguousarray(np.asarray(W_O, f32)[sl, :]).astype(_BF16),
                "bq": np.asarray(b_Q, f32)[sl].reshape(128, 1).copy(),
                "bk": np.asarray(b_K, f32)[sl].reshape(128, 1).copy(),
                "bv": np.asarray(b_V, f32)[sl].reshape(1, 128).astype(_BF16),
            }
        )
    return in_maps


def run_cores(in_maps, **kwargs):
    nc = _get_nc()
    return run_bass_kernel_spmd(nc, in_maps, core_ids=list(range(8)), **kwargs)


def kernel(
    combined_embed, W_K, b_K, W_Q, b_Q, W_V, b_V, W_O, b_O
):  # full inputs -> full output
    in_maps = make_in_maps(
        combined_embed, W_K, b_K, W_Q, b_Q, W_V, b_V, W_O, b_O
    )
    res = run_cores(in_maps)
    out = np.zeros((B, T, D), np.float32)
    for c in range(8):
        out[c // 4] += res.results[c]["out"]
    out += np.asarray(b_O, np.float32)[None, None, :]
    return out
